# revision 14
# baseline (speedup 1.0000x reference)
"""Trainium2 Bass kernel for nn_PolicyNetwork (GRU + MLP head), v2.

Data-parallel over batch (B=256 -> 32 per core, 8 cores). Transposed
on-device layout [feature, batch] throughout.

v2 structure (vs v1):
  - x-projection is computed on the fly into an SBUF ring (no DRAM
    round trip, no serial phase): per 16-step chunk, DMA-transpose the
    x tile, 12 N=512 matmuls + PSUM->SBUF bias/cast copies, interleaved
    between recurrence steps so the PE never stalls.
  - The r/z-gate x-contribution is folded into the recurrence PSUM
    accumulation with one identity-stationary matmul per half, so the
    sigmoid reads PSUM directly (no DVE pre-add).
  - h update uses h_new = n*(1-z) + h*z with zc=1-z and hz=h*z computed
    on the otherwise-idle GPSIMD engine, shortening the post-tanh
    critical path to two DVE ops.
  - Per-half chains with tuned emission order so PE(t+1) k01 matmuls
    overlap the tail of step t's gate chain.
"""

import numpy as np
import ml_dtypes
from contextlib import ExitStack

import concourse.bass as bass
import concourse.bacc as bacc
import concourse.tile as tile
import concourse.mybir as mybir
from concourse.bass_utils import run_bass_kernel_spmd

T, B, D, H, M, A = 512, 256, 256, 512, 512, 64
NCORES = 8
BS = B // NCORES          # 32 batch per core
G = 3 * H                 # 1536 gate width
MCH = G // 128            # 12 gate chunks
KH = H // 128             # 4 hidden chunks
KD = D // 128             # 2 input chunks
MH = M // 128             # 4 mlp chunks
PREF = 16                 # steps per xp chunk
TBC = PREF * BS           # 512 tb-chunk size for x_proj

f32 = mybir.dt.float32
bf16 = mybir.dt.bfloat16
AF = mybir.ActivationFunctionType
ALU = mybir.AluOpType
bf16_np = ml_dtypes.bfloat16

# tuning flags
GP_OFFLOAD = True         # zc / hz on GPSIMD (else DVE)
TAIL1_GP = True           # half-1 q/hn on GPSIMD (else DVE)
COPY_SPLIT = True         # xp copies alternate DVE/ACT (else all DVE)

# xp ring row order: gate chunk m -> row POS[m], so per-half slices are
# contiguous: rows = [r0,r1,z0,z1, r2,r3,z2,z3, n0,n1,n2,n3]
POS = {0: 0, 1: 1, 4: 2, 5: 3, 2: 4, 3: 5, 6: 6, 7: 7,
       8: 8, 9: 9, 10: 10, 11: 11}
# gate-chunk m (natural order) for half p
RZ_M = [[0, 1, 4, 5], [2, 3, 6, 7]]
N_M = [[8, 9], [10, 11]]


def build(nsteps: int = T):
    nc = bacc.Bacc("TRN2", target_bir_lowering=False, debug=False)
    tbn = nsteps * BS
    nchunks = max(1, tbn // TBC)

    x_bf = nc.dram_tensor("x_bf", [tbn, D], bf16, kind="ExternalInput").ap()
    wihT = nc.dram_tensor("wihT", [D, G], bf16, kind="ExternalInput").ap()
    whhT = nc.dram_tensor("whhT", [H, G], bf16, kind="ExternalInput").ap()
    w1T = nc.dram_tensor("w1T", [H, M], bf16, kind="ExternalInput").ap()
    w2T = nc.dram_tensor("w2T", [M, A], bf16, kind="ExternalInput").ap()
    idm = nc.dram_tensor("idm", [128, 128], bf16, kind="ExternalInput").ap()
    bsum = nc.dram_tensor("bsum", [128, MCH], f32, kind="ExternalInput").ap()
    b1T = nc.dram_tensor("b1T", [128, MH], f32, kind="ExternalInput").ap()
    b2c = nc.dram_tensor("b2c", [A, 1], f32, kind="ExternalInput").ap()
    outT = nc.dram_tensor("outT", [A, BS], f32, kind="ExternalOutput").ap()

    with tile.TileContext(nc) as tc, ExitStack() as ctx:
        wpool = ctx.enter_context(tc.tile_pool(name="weights", bufs=1))

        wih_sb = wpool.tile([128, KD, G], bf16, tag="wih")
        for k in range(KD):
            nc.sync.dma_start(wih_sb[:, k, :], wihT[k * 128:(k + 1) * 128, :])
        whh_sb = wpool.tile([128, KH, G], bf16, tag="whh")
        for k in range(KH):
            nc.sync.dma_start(whh_sb[:, k, :], whhT[k * 128:(k + 1) * 128, :])
        w1_sb = wpool.tile([128, KH, M], bf16, tag="w1")
        for k in range(KH):
            nc.sync.dma_start(w1_sb[:, k, :], w1T[k * 128:(k + 1) * 128, :])
        w2_sb = wpool.tile([128, MH, A], bf16, tag="w2")
        for k in range(MH):
            nc.sync.dma_start(w2_sb[:, k, :], w2T[k * 128:(k + 1) * 128, :])
        id_sb = wpool.tile([128, 128], bf16, tag="idm")
        nc.sync.dma_start(id_sb[:], idm[:, :])
        bsum_sb = wpool.tile([128, MCH], f32, tag="bsum")
        nc.sync.dma_start(bsum_sb[:], bsum[:, :])
        b1_sb = wpool.tile([128, MH], f32, tag="b1")
        nc.sync.dma_start(b1_sb[:], b1T[:, :])
        b2_sb = wpool.tile([A, 1], f32, tag="b2")
        nc.sync.dma_start(b2_sb[:], b2c[:, :])

        # pools
        xtpool = ctx.enter_context(tc.tile_pool(name="xT", bufs=2))
        ring = ctx.enter_context(tc.tile_pool(name="xpring", bufs=2))
        xppsum = ctx.enter_context(tc.tile_pool(name="xp_psum", bufs=2, space="PSUM"))
        hpool = ctx.enter_context(tc.tile_pool(name="h", bufs=3))
        gpool = ctx.enter_context(tc.tile_pool(name="gates", bufs=3))
        rpsum = ctx.enter_context(tc.tile_pool(name="rec_psum", bufs=2, space="PSUM"))

        ring_tiles = {}

        def xT_dma(c):
            """DMA-transpose chunk c of x into SBUF."""
            xT = xtpool.tile([128, KD, TBC], bf16, tag="xT")
            for k in range(KD):
                nc.sync.dma_start_transpose(
                    xT[:, k, :],
                    x_bf[c * TBC:(c + 1) * TBC, k * 128:(k + 1) * 128],
                )
            return xT

        def xp_ring_tile(c):
            if c not in ring_tiles:
                ring_tiles[c] = ring.tile(
                    [128, MCH, TBC], bf16, tag="ring", name=f"ring{c}"
                )
            return ring_tiles[c]

        def xp_mchunk(c, m, xT):
            """One gate chunk of x-projection for chunk c -> ring."""
            rt = xp_ring_tile(c)
            ps = xppsum.tile([128, TBC], f32, tag="xps")
            for k in range(KD):
                nc.tensor.matmul(
                    ps[:],
                    wih_sb[:, k, m * 128:(m + 1) * 128],
                    xT[:, k, :],
                    start=(k == 0),
                    stop=(k == KD - 1),
                )
            dst = rt[:, POS[m], :]
            if COPY_SPLIT and (m % 2 == 0):
                nc.scalar.activation(dst, ps[:], AF.Identity,
                                     bias=bsum_sb[:, m:m + 1])
            else:
                nc.vector.tensor_scalar_add(dst, ps[:], bsum_sb[:, m:m + 1])

        # h state tiles: per-half [128, 2, 32]
        h_init0 = hpool.tile([128, 2, BS], bf16, tag="h0")
        h_init1 = hpool.tile([128, 2, BS], bf16, tag="h1")
        h_cur = [h_init0, h_init1]
        nc.vector.memset(h_cur[0][:], 0.0)
        nc.vector.memset(h_cur[1][:], 0.0)

        def rhs_h(k):
            return h_cur[k // 2][:, k % 2, :]

        # prefetch chunk 0 (and its xT): compute fully before first step
        xT_cur = xT_dma(0)
        for m in range(MCH):
            xp_mchunk(0, m, xT_cur)
        xT_next = xT_dma(1) if nchunks > 1 else None

        for t in range(nsteps):
            c = t // PREF
            bi = t % PREF
            xs = slice(bi * BS, (bi + 1) * BS)
            rt = xp_ring_tile(c)

            przn0 = rpsum.tile([128, 6, BS], f32, tag="przn0")
            przn1 = rpsum.tile([128, 6, BS], f32, tag="przn1")
            przn = [przn0, przn1]

            # ---- PE burst + chains, fully per-half pipelined ----
            # Per half p: [id-fold, k01, k23] -> przn[p] stops after only
            # 24 matmuls; chain-0's spine ops are prioritized in the DVE
            # queue (chain-1 has ~1us of slack before k23(t+1) needs h1).
            def mm(p, i, m, k, stop):
                nc.tensor.matmul(
                    przn[p][:, i, :],
                    whh_sb[:, k, m * 128:(m + 1) * 128],
                    rhs_h(k),
                    start=False,
                    stop=stop,
                )

            def mm_half(p):
                nc.tensor.matmul(
                    przn[p][:, 0:4, :], id_sb[:, :], rt[:, 4 * p:4 * p + 4, xs],
                    start=True, stop=False,
                )
                for ks in ((0, 1), (2, 3)):
                    for i, m in enumerate(RZ_M[p]):
                        for k in ks:
                            mm(p, i, m, k, False)
                    for i, m in enumerate(N_M[p]):
                        for k in ks:
                            mm(p, 4 + i, m, k,
                               (ks[0] == 2 and i == len(N_M[p]) - 1 and k == 3))

            rz = [None, None]
            n_g = [None, None]
            zc = [None, None]
            hz = [None, None]
            t2 = [None, None]
            t1 = [None, None]
            eng_zc = nc.gpsimd if GP_OFFLOAD else nc.vector

            def chain_head(p):
                rz[p] = gpool.tile([128, 4, BS], bf16, tag=f"rz{p}", name=f"rz{p}")
                nc.scalar.activation(rz[p][:], przn[p][:, 0:4, :], AF.Sigmoid)
                zc[p] = gpool.tile([128, 2, BS], bf16, tag=f"zc{p}", name=f"zc{p}")
                eng_zc.tensor_scalar(zc[p][:], rz[p][:, 2:4, :], -1.0, 1.0,
                                     ALU.mult, ALU.add)
                hz[p] = gpool.tile([128, 2, BS], bf16, tag=f"hz{p}", name=f"hz{p}")
                eng_zc.tensor_mul(hz[p][:], h_cur[p][:], rz[p][:, 2:4, :])
                t1[p] = gpool.tile([128, 2, BS], bf16, tag=f"t1{p}", name=f"t1{p}")
                nc.vector.tensor_mul(t1[p][:], przn[p][:, 4:6, :], rz[p][:, 0:2, :])
                t2[p] = gpool.tile([128, 2, BS], bf16, tag=f"t2{p}", name=f"t2{p}")
                nc.vector.tensor_add(t2[p][:], t1[p][:], rt[:, 8 + 2 * p:8 + 2 * p + 2, xs])

            def chain_tail(p):
                eng_tail = nc.gpsimd if (TAIL1_GP and p == 1) else nc.vector
                n_g[p] = gpool.tile([128, 2, BS], bf16, tag=f"n{p}", name=f"n{p}")
                nc.scalar.activation(n_g[p][:], t2[p][:], AF.Tanh)
                q = gpool.tile([128, 2, BS], bf16, tag=f"q{p}")
                eng_tail.tensor_mul(q[:], n_g[p][:], zc[p][:])
                hn = hpool.tile([128, 2, BS], bf16, tag=f"h{p}")
                eng_tail.tensor_add(hn[:], q[:], hz[p][:])
                return hn

            h_new = [None, None]
            mm_half(0)
            chain_head(0)
            mm_half(1)
            # chain-1 ACT/GPSIMD heads queue before chain-0's tail so h1
            # lands early; its DVE ops queue after chain-0's tail.
            rz[1] = gpool.tile([128, 4, BS], bf16, tag="rz1", name="rz1")
            nc.scalar.activation(rz[1][:], przn[1][:, 0:4, :], AF.Sigmoid)
            zc[1] = gpool.tile([128, 2, BS], bf16, tag="zc1", name="zc1")
            eng_zc.tensor_scalar(zc[1][:], rz[1][:, 2:4, :], -1.0, 1.0,
                                 ALU.mult, ALU.add)
            hz[1] = gpool.tile([128, 2, BS], bf16, tag="hz1", name="hz1")
            eng_zc.tensor_mul(hz[1][:], h_cur[1][:], rz[1][:, 2:4, :])
            h_new[0] = chain_tail(0)
            t1[1] = gpool.tile([128, 2, BS], bf16, tag="t11", name="t11")
            nc.vector.tensor_mul(t1[1][:], przn[1][:, 4:6, :], rz[1][:, 0:2, :])
            t2[1] = gpool.tile([128, 2, BS], bf16, tag="t21", name="t21")
            nc.vector.tensor_add(t2[1][:], t1[1][:], rt[:, 10:12, xs])
            h_new[1] = chain_tail(1)
            h_cur = h_new

            # interleave next chunk's xp work into this chunk's step slots
            # (PE matmuls fill the idle window while the gate chain runs;
            # copies queue on DVE/ACT after this step's chain ops)
            if bi < MCH and c + 1 < nchunks:
                xp_mchunk(c + 1, bi, xT_next)
            if bi == MCH and c + 2 < nchunks:
                xT_next = xT_dma(c + 2)

        # ---- MLP head ----
        ps_hid = rpsum.tile([128, 6, BS], f32, tag="przn0")
        ps_hid2 = rpsum.tile([128, 6, BS], f32, tag="przn1")
        for mh in range(MH):
            tgt = ps_hid if mh < 2 else ps_hid2
            io = mh % 2
            for k in range(KH):
                nc.tensor.matmul(
                    tgt[:, io, :],
                    w1_sb[:, k, mh * 128:(mh + 1) * 128],
                    rhs_h(k),
                    start=(io == 0 and k == 0),
                    stop=(io == 1 and k == KH - 1),
                )
        hid = gpool.tile([128, MH, BS], bf16, tag="hid")
        for mh in range(MH):
            tgt = ps_hid if mh < 2 else ps_hid2
            nc.scalar.activation(
                hid[:, mh, :], tgt[:, mh % 2, :], AF.Tanh,
                bias=b1_sb[:, mh:mh + 1]
            )
        ps_act = xppsum.tile([128, TBC], f32, tag="xps")
        for k in range(MH):
            nc.tensor.matmul(
                ps_act[0:A, 0:BS],
                w2_sb[:, k, :],
                hid[:, k, :],
                start=(k == 0),
                stop=(k == MH - 1),
            )
        act = gpool.tile([A, BS], f32, tag="act")
        nc.scalar.activation(act[:], ps_act[0:A, 0:BS], AF.Tanh, bias=b2_sb[:, 0:1])
        nc.sync.dma_start(outT[:, :], act[:])

    nc.compile()
    return nc


def prep_inputs(x, W_ih, W_hh, b_ih, b_hh, W1, b1, W2, b2, nsteps: int = T):
    """Host-side prep: transpose/cast weights, shard x over batch."""
    x = np.asarray(x, dtype=np.float32)[:nsteps]
    common = {
        "wihT": np.ascontiguousarray(np.asarray(W_ih).T).astype(bf16_np),
        "whhT": np.ascontiguousarray(np.asarray(W_hh, np.float32).T).astype(bf16_np),
        "w1T": np.ascontiguousarray(np.asarray(W1).T).astype(bf16_np),
        "w2T": np.ascontiguousarray(np.asarray(W2).T).astype(bf16_np),
        "idm": np.eye(128, dtype=bf16_np),
        "bsum": np.ascontiguousarray(
            (np.asarray(b_ih, np.float32) + np.asarray(b_hh, np.float32))
            .reshape(MCH, 128).T),
        "b1T": np.ascontiguousarray(np.asarray(b1, np.float32).reshape(MH, 128).T),
        "b2c": np.ascontiguousarray(np.asarray(b2, np.float32).reshape(A, 1)),
    }
    in_maps = []
    for i in range(NCORES):
        shard = x[:, i * BS:(i + 1) * BS, :].reshape(nsteps * BS, D)
        m = dict(common)
        m["x_bf"] = np.ascontiguousarray(shard).astype(bf16_np)
        in_maps.append(m)
    return in_maps


_CACHE = {}


def run(inputs: dict, nsteps: int = T, trace: bool = False):
    key = nsteps
    if key not in _CACHE:
        _CACHE[key] = build(nsteps)
    nc = _CACHE[key]
    in_maps = prep_inputs(**inputs, nsteps=nsteps)
    res = run_bass_kernel_spmd(
        nc, in_maps, core_ids=list(range(NCORES)), trace=trace
    )
    outs = [r["outT"] for r in res.results]
    full = np.concatenate([o.T for o in outs], axis=0)
    return full.astype(np.float32), res


def kernel(**inputs) -> np.ndarray:
    out, _ = run(inputs)
    return out
